# revision 2
# baseline (speedup 1.0000x reference)
"""Multi-head attention (B=2, S=2048, H=1024, 16 heads) on 8 TRN2 NeuronCores.

Strategy: tensor-parallel over heads (2 heads/core). Each core computes
QKV projections for its heads, full attention, and a partial output
projection; partials are summed on the host.

All matmuls run in float32r (TF32-like fast fp32 mode, 1 cyc/row at
free-dim >= 256). Scores are computed transposed ([t, s]) so softmax's
reduction rides a ones-column appended to V in the AV matmul; the
per-column normalization uses a fast reciprocal + gpsimd partition
broadcast + one DVE multiply.
"""
import sys

for _p in ("/opt/trn_rl_repo", "/root/.axon_site/_ro/trn_rl_repo"):
    if _p not in sys.path:
        sys.path.append(_p)

import numpy as np
import concourse.bass as bass
import concourse.mybir as mybir
import concourse.tile as tile
from concourse import bacc
from concourse.bass_utils import run_bass_kernel_spmd
from concourse.masks import make_identity

N_CORES = 8
B = 2
S = 2048
H = 1024
NH = 16
D = 64                    # head dim
NH_LOC = NH // N_CORES    # 2 heads per core
M_TOT = B * S             # 4096 rows
O_LOC = 3 * NH_LOC * D    # 384 qkv rows per core
DIM_LOC = NH_LOC * D      # 128 out-proj contraction dims per core
SC = 512                  # s-chunk (matmul moving free dim)
TT = 128                  # t-tile (contraction tile in attention)
VSTRIDE = 132             # v_sb per-t-tile block: [v_h0(64) | 1 | v_h1(64) | 1 | pad]

f32 = mybir.dt.float32
f32r = mybir.dt.float32r

_CACHED_NC = None


def build_kernel():
    global _CACHED_NC
    if _CACHED_NC is not None:
        return _CACHED_NC
    nc = bacc.Bacc("TRN2", target_bir_lowering=False, debug=False, num_devices=N_CORES)
    xT = nc.dram_tensor("xT", [H, M_TOT], f32r, kind="ExternalInput").ap()
    wqkvT = nc.dram_tensor("wqkvT", [H, O_LOC], f32r, kind="ExternalInput").ap()
    woutT = nc.dram_tensor("woutT", [DIM_LOC, H], f32r, kind="ExternalInput").ap()
    y = nc.dram_tensor("y", [M_TOT, H], f32, kind="ExternalOutput").ap()

    n_kt = H // 128        # 8 contraction tiles for QKV
    n_sc_tot = M_TOT // SC  # 8 s-chunks across both batches
    n_sc_b = S // SC       # 4 s-chunks per batch
    n_tt = S // TT         # 16 t-tiles per batch
    n_st_b = S // 128      # 16 out-proj s-tiles per batch

    with tile.TileContext(nc) as tc:
        with (
            tc.tile_pool(name="persist", bufs=1) as persist,
            tc.tile_pool(name="xin", bufs=10) as xin,
            tc.tile_pool(name="work", bufs=3) as work,
            tc.tile_pool(name="norm", bufs=4) as norm,
            tc.tile_pool(name="yout", bufs=3) as yout,
        ):
            # ---- persistent SBUF tensors ----
            w_tiles = []
            for k in range(n_kt):
                wt = persist.tile([128, O_LOC], f32r, tag=f"w{k}")
                nc.sync.dma_start(wt[:], wqkvT[k * 128 : (k + 1) * 128, :])
                w_tiles.append(wt)
            wout_t = persist.tile([DIM_LOC, H], f32r, tag="wout")
            nc.sync.dma_start(wout_t[:], woutT[:])

            qT_t = persist.tile([128, M_TOT], f32r, tag="qT")
            kT_t = persist.tile([128, M_TOT], f32r, tag="kT")
            vT_t = persist.tile([128, M_TOT], f32, tag="vT")
            qkv_dst = [qT_t, kT_t, vT_t]

            v_sb = [
                persist.tile([128, n_tt * VSTRIDE], f32r, tag=f"vsb{b}",
                             name=f"v_sb{b}")
                for b in range(B)
            ]
            stacked = [
                persist.tile([128, S], f32r, tag=f"stk{b}", name=f"stacked{b}")
                for b in range(B)
            ]
            ident = persist.tile([128, 128], f32, tag="ident")
            make_identity(nc, ident[:])
            ones_f32 = persist.tile([128, 1], f32, tag="ones")
            nc.vector.memset(ones_f32[:], 1.0)

            # ---- stage 1: QKV projection (out = qkvT [o, s]) ----
            with tc.tile_pool(name="ps_qkv", bufs=2, space="PSUM") as ps_qkv:
                for sc_i in range(n_sc_tot):
                    xts = []
                    for k in range(n_kt):
                        xt = xin.tile([128, SC], f32r, tag="x")
                        nc.sync.dma_start(
                            xt[:],
                            xT[k * 128 : (k + 1) * 128, sc_i * SC : (sc_i + 1) * SC],
                        )
                        xts.append(xt)
                    for ot in range(3):
                        ps = ps_qkv.tile([128, SC], f32, tag="qkv")
                        for k in range(n_kt):
                            nc.tensor.matmul(
                                ps[:],
                                w_tiles[k][:, ot * 128 : (ot + 1) * 128],
                                xts[k][:],
                                start=(k == 0),
                                stop=(k == n_kt - 1),
                            )
                        nc.vector.tensor_copy(
                            qkv_dst[ot][:, sc_i * SC : (sc_i + 1) * SC], ps[:]
                        )

            # ---- stage 2: transpose V to [t, d] with ones columns ----
            with tc.tile_pool(name="ps_tr", bufs=2, space="PSUM") as ps_tr:
                for b in range(B):
                    for tt in range(n_tt):
                        ps = ps_tr.tile([128, 128], f32, tag="tr")
                        nc.tensor.transpose(
                            ps[:],
                            vT_t[:, b * S + tt * TT : b * S + (tt + 1) * TT],
                            ident[:],
                        )
                        base = tt * VSTRIDE
                        nc.vector.tensor_copy(
                            v_sb[b][:, base : base + 64], ps[:, 0:64]
                        )
                        nc.vector.tensor_copy(
                            v_sb[b][:, base + 66 : base + 130], ps[:, 64:128]
                        )
                        nc.vector.tensor_copy(
                            v_sb[b][:, base + 64 : base + 65], ones_f32[:]
                        )
                        nc.vector.tensor_copy(
                            v_sb[b][:, base + 130 : base + 131], ones_f32[:]
                        )

            # ---- stage 3: attention per (b, s-chunk) ----
            with tc.tile_pool(name="ps_att", bufs=1, space="PSUM") as ps_att:
                for b in range(B):
                    for sc_i in range(n_sc_b):
                        s_lo = b * S + sc_i * SC
                        av = [
                            ps_att.tile([128, SC], f32, tag=f"av{h}",
                                        name=f"av{h}_{b}_{sc_i}")
                            for h in range(NH_LOC)
                        ]
                        for tt in range(n_tt):
                            t_lo = b * S + tt * TT
                            for h in range(NH_LOC):
                                d_lo = h * D
                                sc_ps = ps_att.tile([128, SC], f32, tag=f"sc{h}")
                                nc.tensor.matmul(
                                    sc_ps[:],
                                    kT_t[d_lo : d_lo + D, t_lo : t_lo + TT],
                                    qT_t[d_lo : d_lo + D, s_lo : s_lo + SC],
                                    start=True,
                                    stop=True,
                                )
                                expT = work.tile([128, SC], f32r, tag=f"exp{h}")
                                nc.scalar.activation(
                                    expT[:],
                                    sc_ps[:],
                                    mybir.ActivationFunctionType.Exp,
                                    scale=0.125,
                                )
                                vbase = tt * VSTRIDE + h * 66
                                nc.tensor.matmul(
                                    av[h][0:65, :],
                                    v_sb[b][:, vbase : vbase + 65],
                                    expT[:],
                                    start=(tt == 0),
                                    stop=(tt == n_tt - 1),
                                )
                        # normalize: stacked[b][h*64:(h+1)*64, sc] = av[0:64]/av[64]
                        for h in range(NH_LOC):
                            den = norm.tile([1, SC], f32, tag="den")
                            nc.vector.tensor_copy(den[:], av[h][64:65, :])
                            recip = norm.tile([1, SC], f32, tag="recip")
                            nc.vector.reciprocal_approx_fast(recip[:], den[:])
                            bc = norm.tile([64, SC], f32, tag="bc")
                            nc.gpsimd.partition_broadcast(bc[:], recip[:])
                            nc.vector.tensor_mul(
                                stacked[b][
                                    h * D : (h + 1) * D, sc_i * SC : (sc_i + 1) * SC
                                ],
                                av[h][0:64, :],
                                bc[:],
                            )

            # ---- stage 4: out-projection partials ----
            with tc.tile_pool(name="ps_y", bufs=4, space="PSUM") as ps_y:
                for b in range(B):
                    for st in range(n_st_b):
                        y_sb = yout.tile([128, H], f32, tag="y")
                        for oc in range(H // SC):
                            ps = ps_y.tile([128, SC], f32, tag="y")
                            nc.tensor.matmul(
                                ps[:],
                                stacked[b][:, st * 128 : (st + 1) * 128],
                                wout_t[:, oc * SC : (oc + 1) * SC],
                                start=True,
                                stop=True,
                            )
                            nc.vector.tensor_copy(
                                y_sb[:, oc * SC : (oc + 1) * SC], ps[:]
                            )
                        nc.sync.dma_start(
                            y[b * S + st * 128 : b * S + (st + 1) * 128, :], y_sb[:]
                        )
    nc.compile()
    _CACHED_NC = nc
    return nc


def make_in_maps(x, w_qkv, w_out):
    x = np.asarray(x, dtype=np.float32)
    w_qkv = np.asarray(w_qkv, dtype=np.float32)
    w_out = np.asarray(w_out, dtype=np.float32)
    xT = np.ascontiguousarray(x.reshape(M_TOT, H).T)
    in_maps = []
    for c in range(N_CORES):
        rows = slice(c * DIM_LOC, (c + 1) * DIM_LOC)
        wq = w_qkv[0 * H :][rows]
        wk = w_qkv[1 * H :][rows]
        wv = w_qkv[2 * H :][rows]
        wqkvT = np.ascontiguousarray(np.vstack([wq, wk, wv]).T)
        woutT = np.ascontiguousarray(w_out[:, rows].T)
        in_maps.append({"xT": xT, "wqkvT": wqkvT, "woutT": woutT})
    return in_maps


def kernel(x, w_qkv, w_out):
    nc = build_kernel()
    in_maps = make_in_maps(x, w_qkv, w_out)
    res = run_bass_kernel_spmd(nc, in_maps, core_ids=list(range(N_CORES)))
    y = np.zeros((M_TOT, H), dtype=np.float32)
    for c in range(N_CORES):
        y += res.results[c]["y"]
    return y.reshape(B, S, H)


# revision 4
# speedup vs baseline: 1.1589x; 1.1589x over previous
"""Multi-head attention (B=2, S=2048, H=1024, 16 heads) on 8 TRN2 NeuronCores.

Strategy: tensor-parallel over heads (2 heads/core). Each core computes
QKV projections for its heads, full attention, and a partial output
projection; partials are summed on the host.

v2: all matmuls in bf16 (f32 PSUM accumulation). Inputs are converted
to bf16 and pre-transposed on the host; the softmax 1/sqrt(d) scale is
folded into w_k on the host. Scores are computed transposed ([t, s])
with the two local heads packed into one PE pass via row groups
(tile_position), writing one 2-bank PSUM tile so a single ACT exp
instruction covers both heads. Softmax's denominator rides a
ones-column appended to V in the AV matmul; normalization uses a fast
reciprocal + gpsimd partition broadcast + one DVE multiply.
"""
import sys

for _p in ("/opt/trn_rl_repo", "/root/.axon_site/_ro/trn_rl_repo"):
    if _p not in sys.path:
        sys.path.append(_p)

import ml_dtypes
import numpy as np
import concourse.bass as bass
import concourse.mybir as mybir
import concourse.tile as tile
from concourse import bacc
from concourse.bass_utils import run_bass_kernel_spmd
from concourse.masks import make_identity

N_CORES = 8
B = 2
S = 2048
H = 1024
NH = 16
D = 64                    # head dim
NH_LOC = NH // N_CORES    # 2 heads per core
M_TOT = B * S             # 4096 rows
O_LOC = 3 * NH_LOC * D    # 384 qkv rows per core
DIM_LOC = NH_LOC * D      # 128 out-proj contraction dims per core
SC = 512                  # s-chunk (matmul moving free dim)
TT = 128                  # t-tile (contraction tile in attention)
VSTRIDE = 132             # v_sb per-t-tile block: [v_h0(64) | 1 | v_h1(64) | 1 | pad]

f32 = mybir.dt.float32
bf16 = mybir.dt.bfloat16

_CACHED_NC = {}


def build_kernel(reps=1):
    if reps in _CACHED_NC:
        return _CACHED_NC[reps]
    nc = bacc.Bacc("TRN2", target_bir_lowering=False, debug=False, num_devices=N_CORES)
    xT = nc.dram_tensor("xT", [H, M_TOT], bf16, kind="ExternalInput").ap()
    wqkvT = nc.dram_tensor("wqkvT", [H, O_LOC], bf16, kind="ExternalInput").ap()
    woutT = nc.dram_tensor("woutT", [DIM_LOC, H], bf16, kind="ExternalInput").ap()
    y = nc.dram_tensor("y", [M_TOT, H], f32, kind="ExternalOutput").ap()

    n_kt = H // 128        # 8 contraction tiles for QKV
    n_sc_b = S // SC       # 4 s-chunks per batch
    n_tt = S // TT         # 16 t-tiles per batch
    n_st_b = S // 128      # 16 out-proj s-tiles per batch

    with tile.TileContext(nc) as tc:
        with (
            tc.tile_pool(name="persist", bufs=1) as persist,
            tc.tile_pool(name="xin", bufs=10) as xin,
            tc.tile_pool(name="work", bufs=3) as work,
            tc.tile_pool(name="norm", bufs=4) as norm,
            tc.tile_pool(name="yout", bufs=3) as yout,
            # PSUM (8 banks): gen 2 + scpair 2x2 + av0 1 + av1 1
            tc.tile_pool(name="psum", bufs=1, space="PSUM") as psum,
        ):
            # ---- persistent SBUF tensors ----
            w_tiles = []
            for k in range(n_kt):
                wt = persist.tile([128, O_LOC], bf16, tag=f"w{k}")
                nc.sync.dma_start(wt[:], wqkvT[k * 128 : (k + 1) * 128, :])
                w_tiles.append(wt)
            wout_t = persist.tile([DIM_LOC, H], bf16, tag="wout")
            nc.sync.dma_start(wout_t[:], woutT[:])

            # per-batch q/k/v tiles, [dims(128), S] each
            q_t = [persist.tile([128, S], bf16, tag=f"q{b}", name=f"q_t{b}")
                   for b in range(B)]
            k_t = [persist.tile([128, S], bf16, tag=f"k{b}", name=f"k_t{b}")
                   for b in range(B)]
            v_t = [persist.tile([128, S], bf16, tag=f"v{b}", name=f"v_t{b}")
                   for b in range(B)]
            qkv_dst = [q_t, k_t, v_t]

            v_sb = [
                persist.tile([128, n_tt * VSTRIDE], bf16, tag=f"vsb{b}",
                             name=f"v_sb{b}")
                for b in range(B)
            ]
            stacked = [
                persist.tile([128, S], bf16, tag=f"stk{b}", name=f"stacked{b}")
                for b in range(B)
            ]
            ident = persist.tile([128, 128], bf16, tag="ident")
            make_identity(nc, ident[:])

            def body(_iv=None):
                for b in range(B):
                    # ---- stage 1: QKV projection for batch b ----
                    for sc_i in range(n_sc_b):
                        xts = []
                        for k in range(n_kt):
                            xt = xin.tile([128, SC], bf16, tag="x",
                                          name=f"x_{b}_{sc_i}_{k}")
                            nc.sync.dma_start(
                                xt[:],
                                xT[k * 128 : (k + 1) * 128,
                                   b * S + sc_i * SC : b * S + (sc_i + 1) * SC],
                            )
                            xts.append(xt)
                        for ot in range(3):
                            ps = psum.tile([128, SC], f32, tag="gen", bufs=2,
                                           name=f"psqkv_{b}_{sc_i}_{ot}")
                            for k in range(n_kt):
                                nc.tensor.matmul(
                                    ps[:],
                                    w_tiles[k][:, ot * 128 : (ot + 1) * 128],
                                    xts[k][:],
                                    start=(k == 0),
                                    stop=(k == n_kt - 1),
                                )
                            nc.vector.tensor_copy(
                                qkv_dst[ot][b][:, sc_i * SC : (sc_i + 1) * SC],
                                ps[:],
                            )

                    # ---- stage 2: transpose V to [t, d] with ones columns ----
                    for tt in range(n_tt):
                        ps = psum.tile([128, 128], bf16, tag="gen", bufs=2,
                                       name=f"pstr_{b}_{tt}")
                        nc.tensor.transpose(
                            ps[:], v_t[b][:, tt * TT : (tt + 1) * TT], ident[:]
                        )
                        base = tt * VSTRIDE
                        nc.vector.tensor_copy(
                            v_sb[b][:, base : base + 64], ps[:, 0:64]
                        )
                        nc.vector.tensor_copy(
                            v_sb[b][:, base + 66 : base + 130], ps[:, 64:128]
                        )
                        nc.vector.memset(v_sb[b][:, base + 64 : base + 65], 1.0)
                        nc.vector.memset(v_sb[b][:, base + 130 : base + 131], 1.0)

                    # ---- stage 3: attention for batch b ----
                    for sc_i in range(n_sc_b):
                        s_lo = sc_i * SC
                        av = [
                            psum.tile([128, SC], f32, tag=f"av{h}",
                                      name=f"av{h}_{b}_{sc_i}")
                            for h in range(NH_LOC)
                        ]
                        for tt in range(n_tt):
                            t_lo = tt * TT
                            scp = psum.tile([128, 2 * SC], f32, tag="scp", bufs=2,
                                            name=f"scp_{b}_{sc_i}_{tt}")
                            nc.tensor.matmul(
                                scp[:, 0:SC],
                                k_t[b][0:64, t_lo : t_lo + TT],
                                q_t[b][0:64, s_lo : s_lo + SC],
                                start=True, stop=True,
                                tile_position=(0, 0),
                            )
                            nc.tensor.matmul(
                                scp[:, SC : 2 * SC],
                                k_t[b][64:128, t_lo : t_lo + TT],
                                q_t[b][64:128, s_lo : s_lo + SC],
                                start=True, stop=True,
                                tile_position=(64, 0),
                            )
                            expp = work.tile([128, 2 * SC], bf16, tag="expp",
                                             name=f"expp_{b}_{sc_i}_{tt}")
                            nc.scalar.activation(
                                expp[:], scp[:],
                                mybir.ActivationFunctionType.Exp,
                            )
                            vbase = tt * VSTRIDE
                            nc.tensor.matmul(
                                av[0][0:65, :],
                                v_sb[b][:, vbase : vbase + 65],
                                expp[:, 0:SC],
                                start=(tt == 0),
                                stop=(tt == n_tt - 1),
                            )
                            nc.tensor.matmul(
                                av[1][0:65, :],
                                v_sb[b][:, vbase + 66 : vbase + 131],
                                expp[:, SC : 2 * SC],
                                start=(tt == 0),
                                stop=(tt == n_tt - 1),
                            )
                        # normalize: stacked[b][h*64:...] = av[0:64] / av[64]
                        for h in range(NH_LOC):
                            den = norm.tile([1, SC], f32, tag="den",
                                            name=f"den_{b}_{sc_i}_{h}")
                            nc.vector.tensor_copy(den[:], av[h][64:65, :])
                            recip = norm.tile([1, SC], f32, tag="recip",
                                              name=f"recip_{b}_{sc_i}_{h}")
                            nc.vector.reciprocal_approx_fast(recip[:], den[:])
                            bc = norm.tile([64, SC], f32, tag="bc",
                                           name=f"bc_{b}_{sc_i}_{h}")
                            nc.gpsimd.partition_broadcast(bc[:], recip[:])
                            nc.vector.tensor_mul(
                                stacked[b][
                                    h * D : (h + 1) * D, sc_i * SC : (sc_i + 1) * SC
                                ],
                                av[h][0:64, :],
                                bc[:],
                            )

                    # ---- stage 4: out-projection partials for batch b ----
                    for st in range(n_st_b):
                        y_sb = yout.tile([128, H], f32, tag="y",
                                         name=f"y_{b}_{st}")
                        for oc in range(H // SC):
                            ps = psum.tile([128, SC], f32, tag="gen", bufs=2,
                                           name=f"psy_{b}_{st}_{oc}")
                            nc.tensor.matmul(
                                ps[:],
                                stacked[b][:, st * 128 : (st + 1) * 128],
                                wout_t[:, oc * SC : (oc + 1) * SC],
                                start=True,
                                stop=True,
                            )
                            nc.vector.tensor_copy(
                                y_sb[:, oc * SC : (oc + 1) * SC], ps[:]
                            )
                        nc.sync.dma_start(
                            y[b * S + st * 128 : b * S + (st + 1) * 128, :], y_sb[:]
                        )

            if reps == 1:
                body()
            else:
                with tc.For_i(0, reps, 1) as iv:
                    body(iv)
    nc.compile()
    _CACHED_NC[reps] = nc
    return nc


def make_in_maps(x, w_qkv, w_out):
    x = np.asarray(x, dtype=np.float32)
    w_qkv = np.asarray(w_qkv, dtype=np.float32)
    w_out = np.asarray(w_out, dtype=np.float32)
    xT = np.ascontiguousarray(x.reshape(M_TOT, H).T).astype(ml_dtypes.bfloat16)
    scale = 1.0 / np.sqrt(D)
    in_maps = []
    for c in range(N_CORES):
        rows = slice(c * DIM_LOC, (c + 1) * DIM_LOC)
        wq = w_qkv[0 * H :][rows]
        wk = w_qkv[1 * H :][rows] * scale   # fold softmax scale into K
        wv = w_qkv[2 * H :][rows]
        wqkvT = np.ascontiguousarray(np.vstack([wq, wk, wv]).T).astype(
            ml_dtypes.bfloat16
        )
        woutT = np.ascontiguousarray(w_out[:, rows].T).astype(ml_dtypes.bfloat16)
        in_maps.append({"xT": xT, "wqkvT": wqkvT, "woutT": woutT})
    return in_maps


def kernel(x, w_qkv, w_out):
    nc = build_kernel()
    in_maps = make_in_maps(x, w_qkv, w_out)
    res = run_bass_kernel_spmd(nc, in_maps, core_ids=list(range(N_CORES)))
    y = np.zeros((M_TOT, H), dtype=np.float32)
    for c in range(N_CORES):
        y += res.results[c]["y"]
    return y.reshape(B, S, H)
